# revision 18
# baseline (speedup 1.0000x reference)
"""MoE sigmoid routing (DeepSeek-V3 style noaux_tc) on 8 Trainium2 cores.

Full inputs -> shard tokens across 8 cores -> per core (2048 tokens,
16 tiles of 128):

  scores = x @ w.T computed as a two-term precision split:
    hi: fp16(x) @ fp16(w)                          [PE, 1 cyc/row]
    lo: DoubleRow fp8 pair, one instruction stream:
        slot0: e4m3((x-fp16 x)*2^13) @ e4m3(w*2^6)         -> (x_res . w) 2^19
        slot1: e4m3(x*2^6) [ACT-derived] @ e4m3((w-fp16 w)*2^13) -> (x . w_res) 2^19
    scores = hi + 2^-19 * lo
  then sigmoid, group top-2 sums, keep top-4 of 8 groups, top-8 experts
  (vector.max / max_index, exact jax tie order), weights = sigmoid at
  selected experts renormalized * 2.5.

x ships as 3 bytes/elem (fp16 + fp8). The fp8 image of x for slot1 is
produced on the Activation engine (fp16 -> fp8 Copy with scale), so it
costs no DMA. Measured on HW: the kernel is PE-bound; the fp16 hi pass
streams at ~1.7 G cols/s and the fp8 DoubleRow lo pass at ~2.4 G
cols/s (~230us PE per core), with single-ring DMA (~202us) hidden
under it. Alternative arrangements measured and rejected: bf16 hi
(faster at N=512 W-stationary, slower in this N=256 X-stationary
shape), W-stationary 512-token streams (same total: per-MM overhead
scales with stream length), DMA ring-splitting onto the Activation
HWDGE ring (regresses when ACT is busy with the fp8 images).

The kernel is built with target_bir_lowering=True: the walrus-compiled
NEFF path measures ~25% faster end-to-end than the bass_exec path for
this kernel.

Host side only reshapes/quantizes inputs into DMA-friendly layouts and
concatenates outputs.
"""
import numpy as np
import ml_dtypes

import concourse.bacc as bacc
import concourse.mybir as mybir
import concourse.tile as tile
from concourse.bass_utils import run_bass_kernel_spmd

# problem constants (hardcoded per harness contract)
T, DIM, E, G, GW, TOPK = 16384, 7168, 256, 8, 32, 8
N_CORES = 8
T_LOC = T // N_CORES            # 2048 tokens per core
NT = T_LOC // 128               # 16 token tiles per core
KCH = DIM // 128                # 56 k-chunks of 128
ROUTE_SCALE = 2.5

F16 = mybir.dt.float16
F8 = mybir.dt.float8e4
F32 = mybir.dt.float32
NP16 = np.float16

S_HI = 2.0 ** 19                # x16 pre-scale: hi products land at 2^19
S_XLO = 2.0 ** 13               # x residual scale (into e4m3 range)
S_W8 = 2.0 ** 6                 # w fp8 image scale
S_X8 = 2.0 ** -13               # ACT convert scale: x16*2^19 -> e4m3(x*2^6)
S_WLO = 2.0 ** 13               # w residual scale
COMB = 2.0 ** -19               # psum -> score scale on the sigmoid
NEG = -1e30

_CACHE = {}


def _build(lowering: bool = False, gate: bool = False):
    """gate=True (bench only): make every input DMA depend on the previous
    chain iteration's wout output, so back-to-back chained executions fully
    serialize and the chain slope measures single-exec latency."""
    nc = bacc.Bacc(None, target_bir_lowering=lowering)

    x16_d = nc.dram_tensor("x16p", [NT, 128, KCH * 128], F16,
                           kind="ExternalInput")
    xlo_d = nc.dram_tensor("xlop", [NT, 128, KCH * 128], F8,
                           kind="ExternalInput")
    # w tensors split in two k-halves for finer DMA/compute overlap at start
    KH = KCH // 2
    w16a_d = nc.dram_tensor("w16a", [128, KH * E], F16, kind="ExternalInput")
    w16b_d = nc.dram_tensor("w16b", [128, (KCH - KH) * E], F16,
                            kind="ExternalInput")
    w8s1_d = nc.dram_tensor("w8s1", [128, KCH * E], F8,
                            kind="ExternalInput")
    br_d = nc.dram_tensor("brep", [128, E], F32, kind="ExternalInput")
    wout_d = nc.dram_tensor("wout", [128, NT, TOPK], F32,
                            kind="ExternalOutput")
    iout_d = nc.dram_tensor("iout", [128, NT, TOPK], mybir.dt.int32,
                            kind="ExternalOutput")
    tick_d = (nc.dram_tensor("tick", [128, 8], F32, kind="ExternalInput")
              if gate else None)

    with tile.TileContext(nc) as tc:
        with (
            tc.tile_pool(name="wpool", bufs=1) as wpool,
            tc.tile_pool(name="x16pool", bufs=3) as x16pool,
            tc.tile_pool(name="x8pool", bufs=3) as x8pool,
            tc.tile_pool(name="rpool", bufs=2) as rpool,
            tc.tile_pool(name="spool", bufs=2) as spool,
            tc.tile_pool(name="acc", bufs=1) as acc,
            tc.tile_pool(name="psh", bufs=4, space="PSUM") as psh,
        ):
            if gate:
                # tick is threaded from the previous chain iteration's
                # output at the XLA level; gating every input DMA on it
                # serializes back-to-back executions so the chain slope
                # measures single-exec latency (and defeats XLA CSE).
                gt = wpool.tile([128, 8], F32, tag="gate")
                nc.sync.dma_start(out=gt[:], in_=tick_d[:])

                def _gate(view):
                    nc.scalar.activation(
                        view, gt[:], mybir.ActivationFunctionType.Copy,
                        scale=1.0)
            else:
                def _gate(view):
                    pass

            # ---- static tensors ----
            brep = wpool.tile([128, E], F32, tag="brep")
            w16a = wpool.tile([128, KH * E], F16, tag="w16a")
            w16b = wpool.tile([128, (KCH - KH) * E], F16, tag="w16b")
            w8t = wpool.tile([128, 2, KCH * E], F8, tag="w8t")

            # accumulators across tiles
            idxu_all = acc.tile([128, NT * TOPK], mybir.dt.uint32, tag="idxu")
            sv_all = acc.tile([128, NT * TOPK], F32, tag="sv")
            sxu_all = acc.tile([128, NT * TOPK], mybir.dt.uint32, tag="sxu")
            wsel_all = acc.tile([128, NT * TOPK], F32, tag="wsel")

            x16t = [None] * NT
            xpair = [None] * NT

            def issue_load(t):
                x16t[t] = x16pool.tile([128, KCH * 128], F16, tag="x16",
                                       name=f"x16_{t}")
                _gate(x16t[t][:, :8])
                nc.sync.dma_start(out=x16t[t][:], in_=x16_d[t])
                xpair[t] = x8pool.tile([128, 2, KCH * 128], F8, tag="xp",
                                       name=f"xp_{t}")
                _gate(xpair[t][:, 0, :8])
                nc.sync.dma_start(out=xpair[t][:, 0, :], in_=xlo_d[t])
                # fp8 image of x on ACT: slot1 = e4m3(x16 * 2^6)
                nc.scalar.activation(xpair[t][:, 1, :], x16t[t][:],
                                     mybir.ActivationFunctionType.Copy,
                                     scale=S_X8)

            # startup: first x tile before the bulky w tensors; w8 slot0
            # (e4m3(w*2^6)) is derived from w16 on ACT rather than DMAed
            issue_load(0)
            _gate(w16a[:, :8])
            nc.sync.dma_start(out=w16a[:], in_=w16a_d[:])
            nc.scalar.activation(w8t[:, 0, :KH * E], w16a[:],
                                 mybir.ActivationFunctionType.Copy,
                                 scale=S_W8)
            _gate(w16b[:, :8])
            nc.sync.dma_start(out=w16b[:], in_=w16b_d[:])
            nc.scalar.activation(w8t[:, 0, KH * E:], w16b[:],
                                 mybir.ActivationFunctionType.Copy,
                                 scale=S_W8)
            _gate(w8t[:, 1, :8])
            nc.sync.dma_start(out=w8t[:, 1, :], in_=w8s1_d[:])
            _gate(brep[:, :8])
            nc.sync.dma_start(out=brep[:], in_=br_d[:])

            for t in range(NT):
                if t + 1 < NT:
                    issue_load(t + 1)

                # hi and lo accumulate into ONE psum group: hi products are
                # pre-scaled by 2^19 (x16 values carry it), lo DoubleRow
                # products land at 2^13*2^6 = 2^19 as well.
                ps = psh.tile([128, E], F32, tag="ps", name=f"ps_{t}")
                for k in range(KCH):
                    wsl = (w16a[:, k * E:(k + 1) * E] if k < KH
                           else w16b[:, (k - KH) * E:(k - KH + 1) * E])
                    nc.tensor.matmul(
                        ps[:],
                        lhsT=x16t[t][:, k * 128:(k + 1) * 128],
                        rhs=wsl,
                        start=(k == 0), stop=False)
                for k in range(KCH):
                    nc.tensor.matmul(
                        ps[:],
                        lhsT=xpair[t][:, :, k * 128:(k + 1) * 128],
                        rhs=w8t[:, :, k * E:(k + 1) * E],
                        start=False, stop=(k == KCH - 1),
                        perf_mode=mybir.MatmulPerfMode.DoubleRow)

                tg = t
                sig = rpool.tile([128, E], F32, tag="sig")
                nc.scalar.activation(sig[:], ps[:],
                                     mybir.ActivationFunctionType.Sigmoid,
                                     scale=COMB)
                s = rpool.tile([128, E], F32, tag="s")
                nc.vector.tensor_add(s[:], sig[:], brep[:])
                # group top-2 sum
                g1 = spool.tile([128, G], F32, tag="g1")
                nc.vector.reduce_max(
                    g1[:], s[:].rearrange("p (g w) -> p g w", g=G),
                    axis=mybir.AxisListType.X)
                s2 = rpool.tile([128, E], F32, tag="s2")
                nc.vector.match_replace(out=s2[:], in_to_replace=g1[:],
                                        in_values=s[:], imm_value=NEG)
                g2 = spool.tile([128, G], F32, tag="g2")
                nc.vector.reduce_max(
                    g2[:], s2[:].rearrange("p (g w) -> p g w", g=G),
                    axis=mybir.AxisListType.X)
                gsc = spool.tile([128, G], F32, tag="gsc")
                nc.vector.tensor_add(gsc[:], g1[:], g2[:])
                gsort = spool.tile([128, 8], F32, tag="gsort")
                nc.vector.max(out=gsort[:], in_=gsc[:])
                gneg = spool.tile([128, G], F32, tag="gneg")
                nc.vector.tensor_scalar(gneg[:], gsc[:], gsort[:, 3:4],
                                        NEG, op0=mybir.AluOpType.is_lt,
                                        op1=mybir.AluOpType.mult)
                m = rpool.tile([128, E], F32, tag="m")
                nc.vector.tensor_add(
                    m[:].rearrange("p (g w) -> p g w", g=G),
                    s[:].rearrange("p (g w) -> p g w", g=G),
                    gneg[:].to_broadcast([128, G, GW]))
                # top-8 of masked s, in jax tie order
                v = spool.tile([128, 8], F32, tag="v")
                nc.vector.max(out=v[:], in_=m[:])
                nc.vector.max_index(idxu_all[:, tg * 8:(tg + 1) * 8],
                                    v[:], m[:])
                # selected positions -> sigma values, sigma-rank order
                R = rpool.tile([128, E], F32, tag="R")
                nc.vector.match_replace(out=R[:], in_to_replace=v[:],
                                        in_values=m[:], imm_value=NEG)
                selm = rpool.tile([128, E], F32, tag="selm")
                nc.vector.tensor_tensor(selm[:], m[:], R[:],
                                        op=mybir.AluOpType.not_equal)
                sm = rpool.tile([128, E], F32, tag="sm")
                nc.vector.tensor_mul(sm[:], sig[:], selm[:])
                nc.vector.max(out=sv_all[:, tg * 8:(tg + 1) * 8],
                              in_=sm[:])
                nc.vector.max_index(sxu_all[:, tg * 8:(tg + 1) * 8],
                                    sv_all[:, tg * 8:(tg + 1) * 8], sm[:])

                # reorder sigma values into s-rank order via 8x8 id match
                idxf = spool.tile([128, TOPK], F32, tag="idxf")
                nc.vector.tensor_copy(idxf[:], idxu_all[:, tg * 8:(tg + 1) * 8])
                sxf = spool.tile([128, TOPK], F32, tag="sxf")
                nc.vector.tensor_copy(sxf[:], sxu_all[:, tg * 8:(tg + 1) * 8])
                eq = rpool.tile([128, TOPK, TOPK], F32, tag="eq")
                nc.vector.tensor_tensor(
                    eq[:],
                    idxf[:].to_broadcast([128, TOPK, TOPK]),
                    sxf[:].rearrange("p (a j) -> p a j", a=1).to_broadcast(
                        [128, TOPK, TOPK]),
                    op=mybir.AluOpType.is_equal)
                prod = rpool.tile([128, TOPK, TOPK], F32, tag="prod")
                nc.vector.tensor_mul(
                    prod[:], eq[:],
                    sv_all[:, tg * 8:(tg + 1) * 8].rearrange(
                        "p (a j) -> p a j", a=1).to_broadcast(
                        [128, TOPK, TOPK]))
                nc.vector.reduce_sum(
                    wsel_all[:, tg * 8:(tg + 1) * 8].rearrange(
                        "p (a k) -> p a k", a=1),
                    prod[:].rearrange("p k j -> p k j"),
                    axis=mybir.AxisListType.X)

            # tail: renormalize, scale, emit
            rsum = acc.tile([128, NT], F32, tag="rsum")
            nc.vector.reduce_sum(
                rsum[:], wsel_all[:].rearrange("p (t k) -> p t k", t=NT),
                axis=mybir.AxisListType.X)
            rinv = acc.tile([128, NT], F32, tag="rinv")
            nc.vector.reciprocal(rinv[:], rsum[:])
            wdiv = acc.tile([128, NT * TOPK], F32, tag="wdiv")
            nc.vector.tensor_mul(
                wdiv[:].rearrange("p (t k) -> p t k", t=NT),
                wsel_all[:].rearrange("p (t k) -> p t k", t=NT),
                rinv[:].to_broadcast([128, NT, TOPK]))
            wout = acc.tile([128, NT * TOPK], F32, tag="wout")
            nc.vector.tensor_scalar_mul(wout[:], wdiv[:], ROUTE_SCALE)
            iout = acc.tile([128, NT * TOPK], mybir.dt.int32, tag="iout")
            nc.vector.tensor_copy(iout[:], idxu_all[:])
            nc.sync.dma_start(
                out=wout_d[:].rearrange("p t k -> p (t k)"), in_=wout[:])
            nc.sync.dma_start(
                out=iout_d[:].rearrange("p t k -> p (t k)"), in_=iout[:])

    nc.compile()
    return nc


def _tile_layout(a: np.ndarray) -> np.ndarray:
    """[T_LOC, DIM] -> [NT, 128, KCH*128] with out[t, p, k*128+j] =
    a[t*128+j, k*128+p]."""
    v = a.reshape(NT, 128, KCH, 128)            # [t, j, k, p]
    return np.ascontiguousarray(v.transpose(0, 3, 2, 1)).reshape(
        NT, 128, KCH * 128)


def _make_in_maps(x: np.ndarray, weight: np.ndarray, bias: np.ndarray):
    w32 = weight.astype(np.float32)
    w16 = w32.astype(NP16)
    wres = w32 - w16.astype(np.float32)

    def wlay(a, dt):
        # [E, DIM] values -> [128, KCH*E] with out[p, k*E+e] = a[e, k*128+p]
        return np.ascontiguousarray(
            a.T.reshape(KCH, 128, E).transpose(1, 0, 2)).reshape(
            128, KCH * E).astype(dt)

    KH = KCH // 2
    w16p = wlay(w16.astype(np.float32), NP16)
    w16a = np.ascontiguousarray(w16p[:, :KH * E])
    w16b = np.ascontiguousarray(w16p[:, KH * E:])
    w8s1 = wlay(wres * S_WLO, ml_dtypes.float8_e4m3)
    brep = np.ascontiguousarray(
        np.broadcast_to(bias.astype(np.float32), (128, E)))

    maps = []
    for c in range(N_CORES):
        xs = x[c * T_LOC:(c + 1) * T_LOC].astype(np.float32)
        # hi term pre-scaled by 2^19 (exact power-of-2, same rounding)
        xs16 = (xs * S_HI).astype(NP16)
        res = xs - xs16.astype(np.float32) * (1.0 / S_HI)
        xlo8 = (res * S_XLO).astype(ml_dtypes.float8_e4m3)
        maps.append({
            "x16p": _tile_layout(xs16),
            "xlop": _tile_layout(xlo8),
            "w16a": w16a, "w16b": w16b, "w8s1": w8s1, "brep": brep,
        })
    return maps


def kernel(x: np.ndarray, weight: np.ndarray, bias: np.ndarray,
           _trace: bool = False):
    x = np.ascontiguousarray(np.asarray(x, dtype=np.float32))
    weight = np.ascontiguousarray(np.asarray(weight, dtype=np.float32))
    bias = np.ascontiguousarray(np.asarray(bias, dtype=np.float32))

    if "nc" not in _CACHE:
        # lowering=True (NKI path): walrus-compiled NEFF measured ~25%
        # faster end-to-end than the bass_exec path for this kernel.
        _CACHE["nc"] = _build(lowering=True)
    nc = _CACHE["nc"]

    in_maps = _make_in_maps(x, weight, bias)

    kw = {}
    if _trace:
        kw = {"trace": True}
    r = run_bass_kernel_spmd(nc, in_maps, core_ids=list(range(N_CORES)), **kw)
    _CACHE["last_result"] = r

    w_parts, i_parts = [], []
    for c in range(N_CORES):
        wo = r.results[c]["wout"]        # [128, NT, TOPK]
        io = r.results[c]["iout"]
        w_parts.append(wo.transpose(1, 0, 2).reshape(T_LOC, TOPK))
        i_parts.append(io.transpose(1, 0, 2).reshape(T_LOC, TOPK))
    weights_out = np.concatenate(w_parts, axis=0).astype(np.float32)
    indices_out = np.concatenate(i_parts, axis=0).astype(np.int32)
    return weights_out, indices_out



# revision 21
# speedup vs baseline: 1.0955x; 1.0955x over previous
"""MoE sigmoid routing (DeepSeek-V3 style noaux_tc) on 8 Trainium2 cores.

Full inputs -> shard tokens across 8 cores -> per core (2048 tokens,
16 tiles of 128):

  scores = x @ w.T computed as a two-term precision split:
    hi: bf16(x) @ bf16(w)
    lo: DoubleRow fp8 pair, one instruction stream:
        slot0: e4m3((x-bf16 x)*2^13) @ e4m3(w*2^6)         -> (x_res . w) 2^19
        slot1: e4m3(x*2^6) [ACT-derived] @ e4m3((w-bf16 w)*2^13) -> (x . w_res) 2^19
    scores = hi + 2^-19 * lo
  then sigmoid, group top-2 sums, keep top-4 of 8 groups, top-8 experts
  (vector.max / max_index, exact jax tie order), weights = sigmoid at
  selected experts renormalized * 2.5.

x ships as 3 bytes/elem (bf16 + fp8). The fp8 image of x for slot1 is
produced on the Activation engine (bf16 -> fp8 Copy with scale), so it
costs no DMA. Measured on HW: the kernel is PE-bound. bf16 hi measured
237.6us vs 281.0us single-exec with fp16 hi (same gated-chain method);
the coarser bf16 mantissa is absorbed by the fp8 residual pass
(relative error 8.7e-3 on the reference data, threshold 2e-2).
Alternative arrangements measured and rejected: W-stationary 512-token
streams (same total: per-MM overhead scales with stream length), and
DMA ring-splitting onto the Activation HWDGE ring (regresses when ACT
is busy with the fp8 images).

The kernel is built with target_bir_lowering=True: the walrus-compiled
NEFF path measures ~25% faster end-to-end than the bass_exec path for
this kernel.

Host side only reshapes/quantizes inputs into DMA-friendly layouts and
concatenates outputs.
"""
import numpy as np
import ml_dtypes

import concourse.bacc as bacc
import concourse.mybir as mybir
import concourse.tile as tile
from concourse.bass_utils import run_bass_kernel_spmd

# problem constants (hardcoded per harness contract)
T, DIM, E, G, GW, TOPK = 16384, 7168, 256, 8, 32, 8
N_CORES = 8
T_LOC = T // N_CORES            # 2048 tokens per core
NT = T_LOC // 128               # 16 token tiles per core
KCH = DIM // 128                # 56 k-chunks of 128
ROUTE_SCALE = 2.5

F16 = mybir.dt.bfloat16   # hi-pass in bf16: measured 237.6us vs 281.0us
F8 = mybir.dt.float8e4    # with fp16 (same gated-chain method); the fp8
F32 = mybir.dt.float32    # residual pass absorbs the coarser hi mantissa.
NP16 = ml_dtypes.bfloat16

S_HI = 2.0 ** 19                # x16 pre-scale: hi products land at 2^19
S_XLO = 2.0 ** 13               # x residual scale (into e4m3 range)
S_W8 = 2.0 ** 6                 # w fp8 image scale
S_X8 = 2.0 ** -13               # ACT convert scale: x16*2^19 -> e4m3(x*2^6)
S_WLO = 2.0 ** 13               # w residual scale
COMB = 2.0 ** -19               # psum -> score scale on the sigmoid
NEG = -1e30

_CACHE = {}


def _build(lowering: bool = False, gate: bool = False):
    """gate=True (bench only): make every input DMA depend on the previous
    chain iteration's wout output, so back-to-back chained executions fully
    serialize and the chain slope measures single-exec latency."""
    nc = bacc.Bacc(None, target_bir_lowering=lowering)

    x16_d = nc.dram_tensor("x16p", [NT, 128, KCH * 128], F16,
                           kind="ExternalInput")
    xlo_d = nc.dram_tensor("xlop", [NT, 128, KCH * 128], F8,
                           kind="ExternalInput")
    # w tensors split in two k-halves for finer DMA/compute overlap at start
    KH = KCH // 2
    w16a_d = nc.dram_tensor("w16a", [128, KH * E], F16, kind="ExternalInput")
    w16b_d = nc.dram_tensor("w16b", [128, (KCH - KH) * E], F16,
                            kind="ExternalInput")
    w8s1_d = nc.dram_tensor("w8s1", [128, KCH * E], F8,
                            kind="ExternalInput")
    br_d = nc.dram_tensor("brep", [128, E], F32, kind="ExternalInput")
    wout_d = nc.dram_tensor("wout", [128, NT, TOPK], F32,
                            kind="ExternalOutput")
    iout_d = nc.dram_tensor("iout", [128, NT, TOPK], mybir.dt.int32,
                            kind="ExternalOutput")
    tick_d = (nc.dram_tensor("tick", [128, 8], F32, kind="ExternalInput")
              if gate else None)

    with tile.TileContext(nc) as tc:
        with (
            tc.tile_pool(name="wpool", bufs=1) as wpool,
            tc.tile_pool(name="x16pool", bufs=3) as x16pool,
            tc.tile_pool(name="x8pool", bufs=3) as x8pool,
            tc.tile_pool(name="rpool", bufs=2) as rpool,
            tc.tile_pool(name="spool", bufs=2) as spool,
            tc.tile_pool(name="acc", bufs=1) as acc,
            tc.tile_pool(name="psh", bufs=4, space="PSUM") as psh,
        ):
            if gate:
                # tick is threaded from the previous chain iteration's
                # output at the XLA level; gating every input DMA on it
                # serializes back-to-back executions so the chain slope
                # measures single-exec latency (and defeats XLA CSE).
                gt = wpool.tile([128, 8], F32, tag="gate")
                nc.sync.dma_start(out=gt[:], in_=tick_d[:])

                def _gate(view):
                    nc.scalar.activation(
                        view, gt[:], mybir.ActivationFunctionType.Copy,
                        scale=1.0)
            else:
                def _gate(view):
                    pass

            # ---- static tensors ----
            brep = wpool.tile([128, E], F32, tag="brep")
            w16a = wpool.tile([128, KH * E], F16, tag="w16a")
            w16b = wpool.tile([128, (KCH - KH) * E], F16, tag="w16b")
            w8t = wpool.tile([128, 2, KCH * E], F8, tag="w8t")

            # accumulators across tiles
            idxu_all = acc.tile([128, NT * TOPK], mybir.dt.uint32, tag="idxu")
            sv_all = acc.tile([128, NT * TOPK], F32, tag="sv")
            sxu_all = acc.tile([128, NT * TOPK], mybir.dt.uint32, tag="sxu")
            wsel_all = acc.tile([128, NT * TOPK], F32, tag="wsel")

            x16t = [None] * NT
            xpair = [None] * NT

            def issue_load(t):
                x16t[t] = x16pool.tile([128, KCH * 128], F16, tag="x16",
                                       name=f"x16_{t}")
                _gate(x16t[t][:, :8])
                nc.sync.dma_start(out=x16t[t][:], in_=x16_d[t])
                xpair[t] = x8pool.tile([128, 2, KCH * 128], F8, tag="xp",
                                       name=f"xp_{t}")
                _gate(xpair[t][:, 0, :8])
                nc.sync.dma_start(out=xpair[t][:, 0, :], in_=xlo_d[t])
                # fp8 image of x on ACT: slot1 = e4m3(x16 * 2^6)
                nc.scalar.activation(xpair[t][:, 1, :], x16t[t][:],
                                     mybir.ActivationFunctionType.Copy,
                                     scale=S_X8)

            # startup: first x tile before the bulky w tensors; w8 slot0
            # (e4m3(w*2^6)) is derived from w16 on ACT rather than DMAed
            issue_load(0)
            _gate(w16a[:, :8])
            nc.sync.dma_start(out=w16a[:], in_=w16a_d[:])
            nc.scalar.activation(w8t[:, 0, :KH * E], w16a[:],
                                 mybir.ActivationFunctionType.Copy,
                                 scale=S_W8)
            _gate(w16b[:, :8])
            nc.sync.dma_start(out=w16b[:], in_=w16b_d[:])
            nc.scalar.activation(w8t[:, 0, KH * E:], w16b[:],
                                 mybir.ActivationFunctionType.Copy,
                                 scale=S_W8)
            _gate(w8t[:, 1, :8])
            nc.sync.dma_start(out=w8t[:, 1, :], in_=w8s1_d[:])
            _gate(brep[:, :8])
            nc.sync.dma_start(out=brep[:], in_=br_d[:])

            for t in range(NT):
                if t + 1 < NT:
                    issue_load(t + 1)

                # hi and lo accumulate into ONE psum group: hi products are
                # pre-scaled by 2^19 (x16 values carry it), lo DoubleRow
                # products land at 2^13*2^6 = 2^19 as well.
                ps = psh.tile([128, E], F32, tag="ps", name=f"ps_{t}")
                for k in range(KCH):
                    wsl = (w16a[:, k * E:(k + 1) * E] if k < KH
                           else w16b[:, (k - KH) * E:(k - KH + 1) * E])
                    nc.tensor.matmul(
                        ps[:],
                        lhsT=x16t[t][:, k * 128:(k + 1) * 128],
                        rhs=wsl,
                        start=(k == 0), stop=False)
                for k in range(KCH):
                    nc.tensor.matmul(
                        ps[:],
                        lhsT=xpair[t][:, :, k * 128:(k + 1) * 128],
                        rhs=w8t[:, :, k * E:(k + 1) * E],
                        start=False, stop=(k == KCH - 1),
                        perf_mode=mybir.MatmulPerfMode.DoubleRow)

                tg = t
                sig = rpool.tile([128, E], F32, tag="sig")
                nc.scalar.activation(sig[:], ps[:],
                                     mybir.ActivationFunctionType.Sigmoid,
                                     scale=COMB)
                s = rpool.tile([128, E], F32, tag="s")
                nc.vector.tensor_add(s[:], sig[:], brep[:])
                # group top-2 sum
                g1 = spool.tile([128, G], F32, tag="g1")
                nc.vector.reduce_max(
                    g1[:], s[:].rearrange("p (g w) -> p g w", g=G),
                    axis=mybir.AxisListType.X)
                s2 = rpool.tile([128, E], F32, tag="s2")
                nc.vector.match_replace(out=s2[:], in_to_replace=g1[:],
                                        in_values=s[:], imm_value=NEG)
                g2 = spool.tile([128, G], F32, tag="g2")
                nc.vector.reduce_max(
                    g2[:], s2[:].rearrange("p (g w) -> p g w", g=G),
                    axis=mybir.AxisListType.X)
                gsc = spool.tile([128, G], F32, tag="gsc")
                nc.vector.tensor_add(gsc[:], g1[:], g2[:])
                gsort = spool.tile([128, 8], F32, tag="gsort")
                nc.vector.max(out=gsort[:], in_=gsc[:])
                gneg = spool.tile([128, G], F32, tag="gneg")
                nc.vector.tensor_scalar(gneg[:], gsc[:], gsort[:, 3:4],
                                        NEG, op0=mybir.AluOpType.is_lt,
                                        op1=mybir.AluOpType.mult)
                m = rpool.tile([128, E], F32, tag="m")
                nc.vector.tensor_add(
                    m[:].rearrange("p (g w) -> p g w", g=G),
                    s[:].rearrange("p (g w) -> p g w", g=G),
                    gneg[:].to_broadcast([128, G, GW]))
                # top-8 of masked s, in jax tie order
                v = spool.tile([128, 8], F32, tag="v")
                nc.vector.max(out=v[:], in_=m[:])
                nc.vector.max_index(idxu_all[:, tg * 8:(tg + 1) * 8],
                                    v[:], m[:])
                # selected positions -> sigma values, sigma-rank order
                R = rpool.tile([128, E], F32, tag="R")
                nc.vector.match_replace(out=R[:], in_to_replace=v[:],
                                        in_values=m[:], imm_value=NEG)
                selm = rpool.tile([128, E], F32, tag="selm")
                nc.vector.tensor_tensor(selm[:], m[:], R[:],
                                        op=mybir.AluOpType.not_equal)
                sm = rpool.tile([128, E], F32, tag="sm")
                nc.vector.tensor_mul(sm[:], sig[:], selm[:])
                nc.vector.max(out=sv_all[:, tg * 8:(tg + 1) * 8],
                              in_=sm[:])
                nc.vector.max_index(sxu_all[:, tg * 8:(tg + 1) * 8],
                                    sv_all[:, tg * 8:(tg + 1) * 8], sm[:])

                # reorder sigma values into s-rank order via 8x8 id match
                idxf = spool.tile([128, TOPK], F32, tag="idxf")
                nc.vector.tensor_copy(idxf[:], idxu_all[:, tg * 8:(tg + 1) * 8])
                sxf = spool.tile([128, TOPK], F32, tag="sxf")
                nc.vector.tensor_copy(sxf[:], sxu_all[:, tg * 8:(tg + 1) * 8])
                eq = rpool.tile([128, TOPK, TOPK], F32, tag="eq")
                nc.vector.tensor_tensor(
                    eq[:],
                    idxf[:].to_broadcast([128, TOPK, TOPK]),
                    sxf[:].rearrange("p (a j) -> p a j", a=1).to_broadcast(
                        [128, TOPK, TOPK]),
                    op=mybir.AluOpType.is_equal)
                prod = rpool.tile([128, TOPK, TOPK], F32, tag="prod")
                nc.vector.tensor_mul(
                    prod[:], eq[:],
                    sv_all[:, tg * 8:(tg + 1) * 8].rearrange(
                        "p (a j) -> p a j", a=1).to_broadcast(
                        [128, TOPK, TOPK]))
                nc.vector.reduce_sum(
                    wsel_all[:, tg * 8:(tg + 1) * 8].rearrange(
                        "p (a k) -> p a k", a=1),
                    prod[:].rearrange("p k j -> p k j"),
                    axis=mybir.AxisListType.X)

            # tail: renormalize, scale, emit
            rsum = acc.tile([128, NT], F32, tag="rsum")
            nc.vector.reduce_sum(
                rsum[:], wsel_all[:].rearrange("p (t k) -> p t k", t=NT),
                axis=mybir.AxisListType.X)
            rinv = acc.tile([128, NT], F32, tag="rinv")
            nc.vector.reciprocal(rinv[:], rsum[:])
            wdiv = acc.tile([128, NT * TOPK], F32, tag="wdiv")
            nc.vector.tensor_mul(
                wdiv[:].rearrange("p (t k) -> p t k", t=NT),
                wsel_all[:].rearrange("p (t k) -> p t k", t=NT),
                rinv[:].to_broadcast([128, NT, TOPK]))
            wout = acc.tile([128, NT * TOPK], F32, tag="wout")
            nc.vector.tensor_scalar_mul(wout[:], wdiv[:], ROUTE_SCALE)
            iout = acc.tile([128, NT * TOPK], mybir.dt.int32, tag="iout")
            nc.vector.tensor_copy(iout[:], idxu_all[:])
            nc.sync.dma_start(
                out=wout_d[:].rearrange("p t k -> p (t k)"), in_=wout[:])
            nc.sync.dma_start(
                out=iout_d[:].rearrange("p t k -> p (t k)"), in_=iout[:])

    nc.compile()
    return nc


def _tile_layout(a: np.ndarray) -> np.ndarray:
    """[T_LOC, DIM] -> [NT, 128, KCH*128] with out[t, p, k*128+j] =
    a[t*128+j, k*128+p]."""
    v = a.reshape(NT, 128, KCH, 128)            # [t, j, k, p]
    return np.ascontiguousarray(v.transpose(0, 3, 2, 1)).reshape(
        NT, 128, KCH * 128)


def _make_in_maps(x: np.ndarray, weight: np.ndarray, bias: np.ndarray):
    w32 = weight.astype(np.float32)
    w16 = w32.astype(NP16)
    wres = w32 - w16.astype(np.float32)

    def wlay(a, dt):
        # [E, DIM] values -> [128, KCH*E] with out[p, k*E+e] = a[e, k*128+p]
        return np.ascontiguousarray(
            a.T.reshape(KCH, 128, E).transpose(1, 0, 2)).reshape(
            128, KCH * E).astype(dt)

    KH = KCH // 2
    w16p = wlay(w16.astype(np.float32), NP16)
    w16a = np.ascontiguousarray(w16p[:, :KH * E])
    w16b = np.ascontiguousarray(w16p[:, KH * E:])
    w8s1 = wlay(wres * S_WLO, ml_dtypes.float8_e4m3)
    brep = np.ascontiguousarray(
        np.broadcast_to(bias.astype(np.float32), (128, E)))

    maps = []
    for c in range(N_CORES):
        xs = x[c * T_LOC:(c + 1) * T_LOC].astype(np.float32)
        # hi term pre-scaled by 2^19 (exact power-of-2, same rounding)
        xs16 = (xs * S_HI).astype(NP16)
        res = xs - xs16.astype(np.float32) * (1.0 / S_HI)
        xlo8 = (res * S_XLO).astype(ml_dtypes.float8_e4m3)
        maps.append({
            "x16p": _tile_layout(xs16),
            "xlop": _tile_layout(xlo8),
            "w16a": w16a, "w16b": w16b, "w8s1": w8s1, "brep": brep,
        })
    return maps


def kernel(x: np.ndarray, weight: np.ndarray, bias: np.ndarray,
           _trace: bool = False):
    x = np.ascontiguousarray(np.asarray(x, dtype=np.float32))
    weight = np.ascontiguousarray(np.asarray(weight, dtype=np.float32))
    bias = np.ascontiguousarray(np.asarray(bias, dtype=np.float32))

    if "nc" not in _CACHE:
        # lowering=True (NKI path): walrus-compiled NEFF measured ~25%
        # faster end-to-end than the bass_exec path for this kernel.
        _CACHE["nc"] = _build(lowering=True)
    nc = _CACHE["nc"]

    in_maps = _make_in_maps(x, weight, bias)

    kw = {}
    if _trace:
        kw = {"trace": True}
    r = run_bass_kernel_spmd(nc, in_maps, core_ids=list(range(N_CORES)), **kw)
    _CACHE["last_result"] = r

    w_parts, i_parts = [], []
    for c in range(N_CORES):
        wo = r.results[c]["wout"]        # [128, NT, TOPK]
        io = r.results[c]["iout"]
        w_parts.append(wo.transpose(1, 0, 2).reshape(T_LOC, TOPK))
        i_parts.append(io.transpose(1, 0, 2).reshape(T_LOC, TOPK))
    weights_out = np.concatenate(w_parts, axis=0).astype(np.float32)
    indices_out = np.concatenate(i_parts, axis=0).astype(np.int32)
    return weights_out, indices_out



# revision 22
# speedup vs baseline: 1.2328x; 1.1253x over previous
"""MoE sigmoid routing (DeepSeek-V3 style noaux_tc) on 8 Trainium2 cores.

Full inputs -> shard tokens across 8 cores -> per core (2048 tokens,
16 tiles of 128):

  scores = x @ w.T computed as a two-term precision split:
    hi: bf16(x) @ bf16(w)
    lo: DoubleRow fp8 pair, one instruction stream:
        slot0: e4m3((x-bf16 x)*2^13) @ e4m3(w*2^6)         -> (x_res . w) 2^19
        slot1: e4m3(x*2^6) [ACT-derived] @ e4m3((w-bf16 w)*2^13) -> (x . w_res) 2^19
    scores = hi + 2^-19 * lo
  then sigmoid, group top-2 sums, keep top-4 of 8 groups, top-8 experts
  (vector.max / max_index, exact jax tie order), weights = sigmoid at
  selected experts renormalized * 2.5.

x ships as 3 bytes/elem (bf16 + fp8). The fp8 image of x for slot1 is
produced on the Activation engine (bf16 -> fp8 Copy with scale), so it
costs no DMA. Measured on HW: the kernel is PE-bound. bf16 hi measured
237.6us vs 281.0us single-exec with fp16 hi (same gated-chain method);
the coarser bf16 mantissa is absorbed by the fp8 residual pass
(relative error 8.7e-3 on the reference data, threshold 2e-2).
Alternative arrangements measured and rejected: W-stationary 512-token
streams (same total: per-MM overhead scales with stream length), and
DMA ring-splitting onto the Activation HWDGE ring (regresses when ACT
is busy with the fp8 images).

The kernel is built with target_bir_lowering=True: the walrus-compiled
NEFF path measures ~25% faster end-to-end than the bass_exec path for
this kernel.

Host side only reshapes/quantizes inputs into DMA-friendly layouts and
concatenates outputs.
"""
import numpy as np
import ml_dtypes

import concourse.bacc as bacc
import concourse.mybir as mybir
import concourse.tile as tile
from concourse.bass_utils import run_bass_kernel_spmd

# problem constants (hardcoded per harness contract)
T, DIM, E, G, GW, TOPK = 16384, 7168, 256, 8, 32, 8
N_CORES = 8
T_LOC = T // N_CORES            # 2048 tokens per core
NT = T_LOC // 128               # 16 token tiles per core
KCH = DIM // 128                # 56 k-chunks of 128
ROUTE_SCALE = 2.5

F16 = mybir.dt.bfloat16   # hi-pass in bf16: measured 237.6us vs 281.0us
F8 = mybir.dt.float8e4    # with fp16 (same gated-chain method); the fp8
F32 = mybir.dt.float32    # residual pass absorbs the coarser hi mantissa.
NP16 = ml_dtypes.bfloat16

S_HI = 2.0 ** 19                # x16 pre-scale: hi products land at 2^19
S_XLO = 2.0 ** 13               # x residual scale (into e4m3 range)
S_W8 = 2.0 ** 6                 # w fp8 image scale
S_X8 = 2.0 ** -13               # ACT convert scale: x16*2^19 -> e4m3(x*2^6)
S_WLO = 2.0 ** 13               # w residual scale
COMB = 2.0 ** -19               # psum -> score scale on the sigmoid
NEG = -1e30

_CACHE = {}


def _build(lowering: bool = False, gate: bool = False):
    """gate=True (bench only): make every input DMA depend on the previous
    chain iteration's wout output, so back-to-back chained executions fully
    serialize and the chain slope measures single-exec latency."""
    nc = bacc.Bacc(None, target_bir_lowering=lowering)

    x16_d = nc.dram_tensor("x16p", [NT, 128, KCH * 128], F16,
                           kind="ExternalInput")
    xlo_d = nc.dram_tensor("xlop", [NT, 128, KCH * 128], F8,
                           kind="ExternalInput")
    # w tensors split in two k-halves for finer DMA/compute overlap at start
    KH = KCH // 2
    w16a_d = nc.dram_tensor("w16a", [128, KH * E], F16, kind="ExternalInput")
    w16b_d = nc.dram_tensor("w16b", [128, (KCH - KH) * E], F16,
                            kind="ExternalInput")
    w8s1_d = nc.dram_tensor("w8s1", [128, KCH * E], F8,
                            kind="ExternalInput")
    br_d = nc.dram_tensor("brep", [128, E], F32, kind="ExternalInput")
    wout_d = nc.dram_tensor("wout", [128, NT, TOPK], F32,
                            kind="ExternalOutput")
    iout_d = nc.dram_tensor("iout", [128, NT, TOPK], mybir.dt.int32,
                            kind="ExternalOutput")
    tick_d = (nc.dram_tensor("tick", [128, 8], F32, kind="ExternalInput")
              if gate else None)

    with tile.TileContext(nc) as tc:
        with (
            tc.tile_pool(name="wpool", bufs=1) as wpool,
            tc.tile_pool(name="x16pool", bufs=4) as x16pool,
            tc.tile_pool(name="x8pool", bufs=4) as x8pool,
            tc.tile_pool(name="rpool", bufs=2) as rpool,
            tc.tile_pool(name="spool", bufs=2) as spool,
            tc.tile_pool(name="acc", bufs=1) as acc,
            tc.tile_pool(name="psh", bufs=4, space="PSUM") as psh,
        ):
            if gate:
                # tick is threaded from the previous chain iteration's
                # output at the XLA level; gating every input DMA on it
                # serializes back-to-back executions so the chain slope
                # measures single-exec latency (and defeats XLA CSE).
                gt = wpool.tile([128, 8], F32, tag="gate")
                nc.sync.dma_start(out=gt[:], in_=tick_d[:])

                def _gate(view):
                    nc.scalar.activation(
                        view, gt[:], mybir.ActivationFunctionType.Copy,
                        scale=1.0)
            else:
                def _gate(view):
                    pass

            # ---- static tensors ----
            brep = wpool.tile([128, E], F32, tag="brep")
            w16a = wpool.tile([128, KH * E], F16, tag="w16a")
            w16b = wpool.tile([128, (KCH - KH) * E], F16, tag="w16b")
            w8t = wpool.tile([128, 2, KCH * E], F8, tag="w8t")

            # accumulators across tiles
            idxu_all = acc.tile([128, NT * TOPK], mybir.dt.uint32, tag="idxu")
            sv_all = acc.tile([128, NT * TOPK], F32, tag="sv")
            sxu_all = acc.tile([128, NT * TOPK], mybir.dt.uint32, tag="sxu")
            wsel_all = acc.tile([128, NT * TOPK], F32, tag="wsel")

            x16t = [None] * NT
            xpair = [None] * NT

            def issue_load(t):
                x16t[t] = x16pool.tile([128, KCH * 128], F16, tag="x16",
                                       name=f"x16_{t}")
                _gate(x16t[t][:, :8])
                nc.sync.dma_start(out=x16t[t][:], in_=x16_d[t])
                xpair[t] = x8pool.tile([128, 2, KCH * 128], F8, tag="xp",
                                       name=f"xp_{t}")
                _gate(xpair[t][:, 0, :8])
                nc.sync.dma_start(out=xpair[t][:, 0, :], in_=xlo_d[t])
                # fp8 image of x on ACT: slot1 = e4m3(x16 * 2^6)
                nc.scalar.activation(xpair[t][:, 1, :], x16t[t][:],
                                     mybir.ActivationFunctionType.Copy,
                                     scale=S_X8)

            # startup: first x tile before the bulky w tensors; w8 slot0
            # (e4m3(w*2^6)) is derived from w16 on ACT rather than DMAed
            issue_load(0)
            _gate(w16a[:, :8])
            nc.sync.dma_start(out=w16a[:], in_=w16a_d[:])
            nc.scalar.activation(w8t[:, 0, :KH * E], w16a[:],
                                 mybir.ActivationFunctionType.Copy,
                                 scale=S_W8)
            _gate(w16b[:, :8])
            nc.sync.dma_start(out=w16b[:], in_=w16b_d[:])
            nc.scalar.activation(w8t[:, 0, KH * E:], w16b[:],
                                 mybir.ActivationFunctionType.Copy,
                                 scale=S_W8)
            _gate(w8t[:, 1, :8])
            nc.sync.dma_start(out=w8t[:, 1, :], in_=w8s1_d[:])
            _gate(brep[:, :8])
            nc.sync.dma_start(out=brep[:], in_=br_d[:])

            for t in range(NT):
                if t + 1 < NT:
                    issue_load(t + 1)

                # hi and lo accumulate into ONE psum group: hi products are
                # pre-scaled by 2^19 (x16 values carry it), lo DoubleRow
                # products land at 2^13*2^6 = 2^19 as well.
                ps = psh.tile([128, E], F32, tag="ps", name=f"ps_{t}")
                for k in range(KCH):
                    wsl = (w16a[:, k * E:(k + 1) * E] if k < KH
                           else w16b[:, (k - KH) * E:(k - KH + 1) * E])
                    nc.tensor.matmul(
                        ps[:],
                        lhsT=x16t[t][:, k * 128:(k + 1) * 128],
                        rhs=wsl,
                        start=(k == 0), stop=False)
                for k in range(KCH):
                    nc.tensor.matmul(
                        ps[:],
                        lhsT=xpair[t][:, :, k * 128:(k + 1) * 128],
                        rhs=w8t[:, :, k * E:(k + 1) * E],
                        start=False, stop=(k == KCH - 1),
                        perf_mode=mybir.MatmulPerfMode.DoubleRow)

                tg = t
                sig = rpool.tile([128, E], F32, tag="sig")
                nc.scalar.activation(sig[:], ps[:],
                                     mybir.ActivationFunctionType.Sigmoid,
                                     scale=COMB)
                s = rpool.tile([128, E], F32, tag="s")
                nc.vector.tensor_add(s[:], sig[:], brep[:])
                # group top-2 sum
                g1 = spool.tile([128, G], F32, tag="g1")
                nc.vector.reduce_max(
                    g1[:], s[:].rearrange("p (g w) -> p g w", g=G),
                    axis=mybir.AxisListType.X)
                s2 = rpool.tile([128, E], F32, tag="s2")
                nc.vector.match_replace(out=s2[:], in_to_replace=g1[:],
                                        in_values=s[:], imm_value=NEG)
                g2 = spool.tile([128, G], F32, tag="g2")
                nc.vector.reduce_max(
                    g2[:], s2[:].rearrange("p (g w) -> p g w", g=G),
                    axis=mybir.AxisListType.X)
                gsc = spool.tile([128, G], F32, tag="gsc")
                nc.vector.tensor_add(gsc[:], g1[:], g2[:])
                gsort = spool.tile([128, 8], F32, tag="gsort")
                nc.vector.max(out=gsort[:], in_=gsc[:])
                gneg = spool.tile([128, G], F32, tag="gneg")
                nc.vector.tensor_scalar(gneg[:], gsc[:], gsort[:, 3:4],
                                        NEG, op0=mybir.AluOpType.is_lt,
                                        op1=mybir.AluOpType.mult)
                m = rpool.tile([128, E], F32, tag="m")
                nc.vector.tensor_add(
                    m[:].rearrange("p (g w) -> p g w", g=G),
                    s[:].rearrange("p (g w) -> p g w", g=G),
                    gneg[:].to_broadcast([128, G, GW]))
                # top-8 of masked s, in jax tie order
                v = spool.tile([128, 8], F32, tag="v")
                nc.vector.max(out=v[:], in_=m[:])
                nc.vector.max_index(idxu_all[:, tg * 8:(tg + 1) * 8],
                                    v[:], m[:])
                # selected positions -> sigma values, sigma-rank order
                R = rpool.tile([128, E], F32, tag="R")
                nc.vector.match_replace(out=R[:], in_to_replace=v[:],
                                        in_values=m[:], imm_value=NEG)
                selm = rpool.tile([128, E], F32, tag="selm")
                nc.vector.tensor_tensor(selm[:], m[:], R[:],
                                        op=mybir.AluOpType.not_equal)
                sm = rpool.tile([128, E], F32, tag="sm")
                nc.vector.tensor_mul(sm[:], sig[:], selm[:])
                nc.vector.max(out=sv_all[:, tg * 8:(tg + 1) * 8],
                              in_=sm[:])
                nc.vector.max_index(sxu_all[:, tg * 8:(tg + 1) * 8],
                                    sv_all[:, tg * 8:(tg + 1) * 8], sm[:])

                # reorder sigma values into s-rank order via 8x8 id match
                idxf = spool.tile([128, TOPK], F32, tag="idxf")
                nc.vector.tensor_copy(idxf[:], idxu_all[:, tg * 8:(tg + 1) * 8])
                sxf = spool.tile([128, TOPK], F32, tag="sxf")
                nc.vector.tensor_copy(sxf[:], sxu_all[:, tg * 8:(tg + 1) * 8])
                eq = rpool.tile([128, TOPK, TOPK], F32, tag="eq")
                nc.vector.tensor_tensor(
                    eq[:],
                    idxf[:].to_broadcast([128, TOPK, TOPK]),
                    sxf[:].rearrange("p (a j) -> p a j", a=1).to_broadcast(
                        [128, TOPK, TOPK]),
                    op=mybir.AluOpType.is_equal)
                prod = rpool.tile([128, TOPK, TOPK], F32, tag="prod")
                nc.vector.tensor_mul(
                    prod[:], eq[:],
                    sv_all[:, tg * 8:(tg + 1) * 8].rearrange(
                        "p (a j) -> p a j", a=1).to_broadcast(
                        [128, TOPK, TOPK]))
                nc.vector.reduce_sum(
                    wsel_all[:, tg * 8:(tg + 1) * 8].rearrange(
                        "p (a k) -> p a k", a=1),
                    prod[:].rearrange("p k j -> p k j"),
                    axis=mybir.AxisListType.X)

            # tail: renormalize, scale, emit
            rsum = acc.tile([128, NT], F32, tag="rsum")
            nc.vector.reduce_sum(
                rsum[:], wsel_all[:].rearrange("p (t k) -> p t k", t=NT),
                axis=mybir.AxisListType.X)
            rinv = acc.tile([128, NT], F32, tag="rinv")
            nc.vector.reciprocal(rinv[:], rsum[:])
            wdiv = acc.tile([128, NT * TOPK], F32, tag="wdiv")
            nc.vector.tensor_mul(
                wdiv[:].rearrange("p (t k) -> p t k", t=NT),
                wsel_all[:].rearrange("p (t k) -> p t k", t=NT),
                rinv[:].to_broadcast([128, NT, TOPK]))
            wout = acc.tile([128, NT * TOPK], F32, tag="wout")
            nc.vector.tensor_scalar_mul(wout[:], wdiv[:], ROUTE_SCALE)
            iout = acc.tile([128, NT * TOPK], mybir.dt.int32, tag="iout")
            nc.vector.tensor_copy(iout[:], idxu_all[:])
            nc.sync.dma_start(
                out=wout_d[:].rearrange("p t k -> p (t k)"), in_=wout[:])
            nc.sync.dma_start(
                out=iout_d[:].rearrange("p t k -> p (t k)"), in_=iout[:])

    nc.compile()
    return nc


def _tile_layout(a: np.ndarray) -> np.ndarray:
    """[T_LOC, DIM] -> [NT, 128, KCH*128] with out[t, p, k*128+j] =
    a[t*128+j, k*128+p]."""
    v = a.reshape(NT, 128, KCH, 128)            # [t, j, k, p]
    return np.ascontiguousarray(v.transpose(0, 3, 2, 1)).reshape(
        NT, 128, KCH * 128)


def _make_in_maps(x: np.ndarray, weight: np.ndarray, bias: np.ndarray):
    w32 = weight.astype(np.float32)
    w16 = w32.astype(NP16)
    wres = w32 - w16.astype(np.float32)

    def wlay(a, dt):
        # [E, DIM] values -> [128, KCH*E] with out[p, k*E+e] = a[e, k*128+p]
        return np.ascontiguousarray(
            a.T.reshape(KCH, 128, E).transpose(1, 0, 2)).reshape(
            128, KCH * E).astype(dt)

    KH = KCH // 2
    w16p = wlay(w16.astype(np.float32), NP16)
    w16a = np.ascontiguousarray(w16p[:, :KH * E])
    w16b = np.ascontiguousarray(w16p[:, KH * E:])
    w8s1 = wlay(wres * S_WLO, ml_dtypes.float8_e4m3)
    brep = np.ascontiguousarray(
        np.broadcast_to(bias.astype(np.float32), (128, E)))

    maps = []
    for c in range(N_CORES):
        xs = x[c * T_LOC:(c + 1) * T_LOC].astype(np.float32)
        # hi term pre-scaled by 2^19 (exact power-of-2, same rounding)
        xs16 = (xs * S_HI).astype(NP16)
        res = xs - xs16.astype(np.float32) * (1.0 / S_HI)
        xlo8 = (res * S_XLO).astype(ml_dtypes.float8_e4m3)
        maps.append({
            "x16p": _tile_layout(xs16),
            "xlop": _tile_layout(xlo8),
            "w16a": w16a, "w16b": w16b, "w8s1": w8s1, "brep": brep,
        })
    return maps


def kernel(x: np.ndarray, weight: np.ndarray, bias: np.ndarray,
           _trace: bool = False):
    x = np.ascontiguousarray(np.asarray(x, dtype=np.float32))
    weight = np.ascontiguousarray(np.asarray(weight, dtype=np.float32))
    bias = np.ascontiguousarray(np.asarray(bias, dtype=np.float32))

    if "nc" not in _CACHE:
        # lowering=True (NKI path): walrus-compiled NEFF measured ~25%
        # faster end-to-end than the bass_exec path for this kernel.
        _CACHE["nc"] = _build(lowering=True)
    nc = _CACHE["nc"]

    in_maps = _make_in_maps(x, weight, bias)

    kw = {}
    if _trace:
        kw = {"trace": True}
    r = run_bass_kernel_spmd(nc, in_maps, core_ids=list(range(N_CORES)), **kw)
    _CACHE["last_result"] = r

    w_parts, i_parts = [], []
    for c in range(N_CORES):
        wo = r.results[c]["wout"]        # [128, NT, TOPK]
        io = r.results[c]["iout"]
        w_parts.append(wo.transpose(1, 0, 2).reshape(T_LOC, TOPK))
        i_parts.append(io.transpose(1, 0, 2).reshape(T_LOC, TOPK))
    weights_out = np.concatenate(w_parts, axis=0).astype(np.float32)
    indices_out = np.concatenate(i_parts, axis=0).astype(np.int32)
    return weights_out, indices_out

